# revision 9
# baseline (speedup 1.0000x reference)
"""Causal multi-head attention mixer on 8 TRN2 NeuronCores.

Problem: x[2, 2048, 1024], 16 heads x 64 dim, RoPE, causal softmax, Wo proj.

Sharding (host-side): core c -> (batch b = c//4, head-group hg = c%4 of 4
heads = 256 channels). Each core computes its 4 heads' attention for its
batch and a partial output projection over its 256 Wo columns; the host
sums the 4 partials per batch. No on-device collectives.

Device kernel (per core), all matmuls in float32r (full PE rate, ~1.5e-4
rounding):
  - QKV projections from xT [1024, 2048] streamed in seq-chunks of 512,
    accumulating over the 8 K-tiles in PSUM.
  - RoPE applied on PSUM eviction of q/k (transposed layout [dim, seq]):
    out = q*cos + swap32(q*sin_signed), the 32-row block swap done with
    SBUF->SBUF DMAs.
  - Attention in transposed layout: scores.T [k-tile 128, q-block 512] =
    kT.T @ qT on PE; exp on ACT (scale=1/8) straight PSUM->SBUF (f32r);
    causal mask = one [128,128] triangular multiply on diagonal subtiles;
    P.T @ V via PE with a ones-column appended to V so the softmax
    denominators accumulate for free as row 64 of the output.
  - Normalize: gpsimd partition-broadcast of the denominator row, DVE
    divide, writing lhsT tiles for the output projection.
  - Output projection vs WoT [256, 1024], fp32 copy-back, DMA out.
"""

import numpy as np
from contextlib import ExitStack

import concourse.bass as bass
import concourse.tile as tile
from concourse import bacc, mybir
from concourse.bass_utils import run_bass_kernel_spmd

F32 = mybir.dt.float32
F32R = mybir.dt.float32r
AOP = mybir.AluOpType
AF = mybir.ActivationFunctionType

S = 2048          # seq len
DM = 1024         # model dim
HPC = 4           # heads per core
DH = 64           # head dim
CH = HPC * DH     # channels per core = 256
NCH = 4           # seq chunks (of 512) for projections / q-blocks
QB = S // NCH     # 512
KT = 128          # k tile
NKT = S // KT     # 16
ROPE_PERIOD = 10000.0


def _rope_tables():
    inv_freq = 1.0 / (ROPE_PERIOD ** (np.arange(0, DH, 2, dtype=np.float64) / DH))
    t = np.arange(S, dtype=np.float64)
    freqs = np.outer(inv_freq, t)           # [32, S]
    cos32 = np.cos(freqs).astype(np.float32)
    sin32 = np.sin(freqs).astype(np.float32)
    # cosT rows d: cos(s * invf[d % 32]), duplicated across the two
    # 64-row head slots of a head-pair tile.
    cos64 = np.concatenate([cos32, cos32], axis=0)          # [64, S]
    cosT = np.concatenate([cos64, cos64], axis=0)           # [128, S]
    # sin_signed2[d] multiplies q[d] and lands (after the 32-block swap)
    # on row d^32: rows 0:32 -> +sin (feeds out[32:64]), rows 32:64 -> -sin
    # (feeds out[0:32]).
    sin64 = np.concatenate([sin32, -sin32], axis=0)         # [64, S]
    sinT2 = np.concatenate([sin64, sin64], axis=0)          # [128, S]
    return cosT, sinT2


def _build():
    nc = bacc.Bacc(None, target_bir_lowering=False)

    xT_ext = nc.dram_tensor("xT", [DM, S], F32R, kind="ExternalInput")
    wqT_ext = nc.dram_tensor("wqT", [DM, CH], F32R, kind="ExternalInput")
    wkT_ext = nc.dram_tensor("wkT", [DM, CH], F32R, kind="ExternalInput")
    wvT_ext = nc.dram_tensor("wvT", [DM, CH], F32R, kind="ExternalInput")
    woT_ext = nc.dram_tensor("woT", [CH, DM], F32R, kind="ExternalInput")
    out_ext = nc.dram_tensor("out", [S, DM], F32, kind="ExternalOutput")

    cosT_np, sinT2_np = _rope_tables()
    cosT_dram = nc.inline_tensor(cosT_np, name="cosT")
    sinT2_dram = nc.inline_tensor(sinT2_np, name="sinT2")
    tri_np = (np.arange(KT)[:, None] <= np.arange(KT)[None, :]).astype(np.float32)
    tri_dram = nc.inline_tensor(tri_np, name="tri")

    with tile.TileContext(nc) as tc, ExitStack() as ctx:
        const = ctx.enter_context(tc.tile_pool(name="const", bufs=1))
        persist = ctx.enter_context(tc.tile_pool(name="persist", bufs=1))

        cosT = const.tile([128, S], F32, tag="cosT")
        sinT2 = const.tile([128, S], F32, tag="sinT2")
        tri = const.tile([KT, KT], F32, tag="tri")
        nc.sync.dma_start(cosT[:], cosT_dram[:])
        nc.sync.dma_start(sinT2[:], sinT2_dram[:])
        nc.sync.dma_start(tri[:], tri_dram[:])

        wq_t = [const.tile([128, CH], F32R, name=f"wq{k}") for k in range(8)]
        wk_t = [const.tile([128, CH], F32R, name=f"wk{k}") for k in range(8)]
        wv_t = [const.tile([128, CH], F32R, name=f"wv{k}") for k in range(8)]
        wo_t = [const.tile([128, DM], F32R, name=f"wo{k}") for k in range(2)]
        for k in range(8):
            nc.sync.dma_start(wq_t[k][:], wqT_ext[128 * k:128 * (k + 1), :])
            nc.sync.dma_start(wk_t[k][:], wkT_ext[128 * k:128 * (k + 1), :])
            nc.sync.dma_start(wv_t[k][:], wvT_ext[128 * k:128 * (k + 1), :])
        for k in range(2):
            nc.sync.dma_start(wo_t[k][:], woT_ext[128 * k:128 * (k + 1), :])

        # persistent activations (transposed layouts, head-pair tiles)
        qT_sb = [persist.tile([128, S], F32R, name=f"qT{m}") for m in range(2)]
        kT_sb = [persist.tile([128, S], F32R, name=f"kT{m}") for m in range(2)]
        attn_sb = [persist.tile([128, S], F32R, name=f"at{m}") for m in range(2)]
        v_sb = [persist.tile([128, HPC * (DH + 1)], F32R, name=f"v{k}")
                for k in range(NKT)]

        # ---------------- phase 1: QKV projections + RoPE ----------------
        with ExitStack() as pctx:
            xpool = pctx.enter_context(tc.tile_pool(name="xpool", bufs=4))
            ppool = pctx.enter_context(
                tc.tile_pool(name="ppool", bufs=1, space="PSUM"))
            rpool = pctx.enter_context(tc.tile_pool(name="rpool", bufs=3))

            for cn in range(NCH):
                cs = slice(QB * cn, QB * (cn + 1))
                q_ps = [ppool.tile([128, QB], F32, name=f"qp{m}", tag=f"qp{m}") for m in range(2)]
                k_ps = [ppool.tile([128, QB], F32, name=f"kp{m}", tag=f"kp{m}") for m in range(2)]
                v_ps = [ppool.tile([128, CH], F32, name=f"vp{sq}", tag=f"vp{sq}") for sq in range(4)]
                for kt in range(8):
                    xt = xpool.tile([128, QB], F32R, tag="xt")
                    nc.scalar.dma_start(xt[:], xT_ext[128 * kt:128 * (kt + 1), cs])
                    st, sp = kt == 0, kt == 7
                    for m in range(2):
                        nc.tensor.matmul(q_ps[m][:], wq_t[kt][:, 128 * m:128 * (m + 1)],
                                         xt[:], start=st, stop=sp)
                        nc.tensor.matmul(k_ps[m][:], wk_t[kt][:, 128 * m:128 * (m + 1)],
                                         xt[:], start=st, stop=sp)
                    for sq in range(4):
                        nc.tensor.matmul(v_ps[sq][:], xt[:, 128 * sq:128 * (sq + 1)],
                                         wv_t[kt][:], start=st, stop=sp)

                # RoPE eviction for q and k head-pair tiles
                for ps_list, dst in ((q_ps, qT_sb), (k_ps, kT_sb)):
                    for m in range(2):
                        t_t = rpool.tile([128, QB], F32, tag="ropet")
                        r_t = rpool.tile([128, QB], F32, tag="roper")
                        rs_t = rpool.tile([128, QB], F32, tag="ropes")
                        nc.vector.tensor_tensor(
                            t_t[:], ps_list[m][:], cosT[:, cs], AOP.mult)
                        nc.vector.tensor_tensor(
                            r_t[:], ps_list[m][:], sinT2[:, cs], AOP.mult)
                        for blk, eng in zip(range(4), (nc.sync, nc.gpsimd,
                                                       nc.scalar, nc.sync)):
                            src = slice(32 * (blk ^ 1), 32 * (blk ^ 1) + 32)
                            dst_sl = slice(32 * blk, 32 * blk + 32)
                            eng.dma_start(rs_t[dst_sl, :], r_t[src, :])
                        nc.vector.tensor_tensor(
                            dst[m][:, cs], t_t[:], rs_t[:], AOP.add)

                # V eviction: interleave 4 heads with a ones column each
                for sq in range(4):
                    vt = v_sb[4 * cn + sq]
                    nc.vector.memset(vt[:].bitcast(F32), 1.0)
                    nc.vector.tensor_copy(
                        vt[:].rearrange("p (h e) -> p h e", h=HPC)[:, :, 0:64],
                        v_ps[sq][:].rearrange("p (h d) -> p h d", h=HPC))

        # ---------------- phase 2: attention + out-projection ----------------
        with ExitStack() as actx:
            scpool = actx.enter_context(
                tc.tile_pool(name="scpool", bufs=3, space="PSUM"))
            popool = actx.enter_context(
                tc.tile_pool(name="popool", bufs=3, space="PSUM"))
            ptpool = actx.enter_context(tc.tile_pool(name="ptpool", bufs=4))
            nrmpool = actx.enter_context(tc.tile_pool(name="nrmpool", bufs=2))
            opool = actx.enter_context(
                tc.tile_pool(name="opool", bufs=2, space="PSUM"))
            ospool = actx.enter_context(tc.tile_pool(name="ospool", bufs=3))

            for b in range(NCH):
                qs = slice(QB * b, QB * (b + 1))
                for h in range(HPC):
                    m, hh = divmod(h, 2)
                    hrow = slice(64 * hh, 64 * hh + 64)
                    po = popool.tile([DH + 1, QB], F32, tag="po")
                    nkt = 4 * b + 4
                    for ki in range(nkt):
                        d = ki - 4 * b
                        qlo = max(0, 128 * d)
                        sc = scpool.tile([128, QB], F32, tag="sc")
                        nc.tensor.matmul(
                            sc[:, qlo:QB],
                            kT_sb[m][hrow, 128 * ki:128 * (ki + 1)],
                            qT_sb[m][hrow, QB * b + qlo:QB * (b + 1)],
                            start=True, stop=True)
                        pt = ptpool.tile([128, QB], F32R, tag="pt")
                        nc.scalar.activation(pt[:, qlo:QB], sc[:, qlo:QB],
                                             AF.Exp, scale=0.125)
                        if d >= 0:
                            nc.gpsimd.tensor_tensor(
                                pt[:, qlo:qlo + 128],
                                pt[:, qlo:qlo + 128].bitcast(F32),
                                tri[:], AOP.mult)
                        nc.tensor.matmul(
                            po[:, qlo:QB],
                            v_sb[ki][:, 65 * h:65 * (h + 1)],
                            pt[:, qlo:QB],
                            start=(ki == 0), stop=(ki == nkt - 1))
                    # normalize by the accumulated denominator row
                    rsum = nrmpool.tile([1, QB], F32, tag="rsum")
                    rbc = nrmpool.tile([DH, QB], F32, tag="rbc")
                    nc.vector.reciprocal(rsum[:], po[DH:DH + 1, :])
                    nc.gpsimd.partition_broadcast(rbc[:], rsum[:])
                    nc.vector.tensor_tensor(
                        attn_sb[m][hrow, qs], po[0:DH, :], rbc[:], AOP.mult)

                # out-projection for the 4 seq tiles of this block
                for sq in range(4 * b, 4 * b + 4):
                    ssl = slice(128 * sq, 128 * (sq + 1))
                    ot = ospool.tile([128, DM], F32, tag="ot")
                    for on in range(2):
                        osl = slice(512 * on, 512 * (on + 1))
                        ops = opool.tile([128, 512], F32, tag="ops")
                        for ct in range(2):
                            nc.tensor.matmul(ops[:], attn_sb[ct][:, ssl],
                                             wo_t[ct][:, osl],
                                             start=(ct == 0), stop=(ct == 1))
                        nc.scalar.copy(ot[:, osl], ops[:])
                    nc.gpsimd.dma_start(out_ext[ssl, :], ot[:])

    nc.compile()
    return nc


_NC_CACHE = []


def kernel(x, Wq, Wk, Wv, Wo):
    x = np.asarray(x, dtype=np.float32)
    Wq = np.asarray(Wq, dtype=np.float32)
    Wk = np.asarray(Wk, dtype=np.float32)
    Wv = np.asarray(Wv, dtype=np.float32)
    Wo = np.asarray(Wo, dtype=np.float32)

    in_maps = []
    for c in range(8):
        b, hg = divmod(c, 4)
        rows = slice(CH * hg, CH * (hg + 1))
        in_maps.append({
            "xT": np.ascontiguousarray(x[b].T),
            "wqT": np.ascontiguousarray(Wq[rows, :].T),
            "wkT": np.ascontiguousarray(Wk[rows, :].T),
            "wvT": np.ascontiguousarray(Wv[rows, :].T),
            "woT": np.ascontiguousarray(Wo[:, rows].T),
        })

    if not _NC_CACHE:
        _NC_CACHE.append(_build())
    nc = _NC_CACHE[0]

    res = run_bass_kernel_spmd(nc, in_maps, list(range(8)))
    out = np.zeros((2, S, DM), dtype=np.float32)
    for c in range(8):
        out[c // 4] += res.results[c]["out"]
    return out


# revision 10
# speedup vs baseline: 1.2623x; 1.2623x over previous
"""Causal multi-head attention mixer on 8 TRN2 NeuronCores.

Problem: x[2, 2048, 1024], 16 heads x 64 dim, RoPE, causal softmax, Wo proj.

Sharding (host-side): core c -> (batch b = c//4, head-group hg = c%4 of 4
heads = 256 channels). Each core computes its 4 heads' attention for its
batch and a partial output projection over its 256 Wo columns; the host
sums the 4 partials per batch. No on-device collectives.

Device kernel (per core), all matmuls in float32r (full PE rate, ~1.5e-4
rounding):
  - QKV projections from xT [1024, 2048] streamed in seq-chunks of 512,
    accumulating over the 8 K-tiles in PSUM.
  - RoPE applied on PSUM eviction of q/k (transposed layout [dim, seq]):
    out = q*cos + swap32(q*sin_signed), the 32-row block swap done with
    SBUF->SBUF DMAs.
  - Attention in transposed layout: scores.T [k-tile 128, q-block 512] =
    kT.T @ qT on PE; exp on ACT (scale=1/8) straight PSUM->SBUF (f32r);
    causal mask = one [128,128] triangular multiply on diagonal subtiles;
    P.T @ V via PE with a ones-column appended to V so the softmax
    denominators accumulate for free as row 64 of the output.
  - Normalize: gpsimd partition-broadcast of the denominator row, DVE
    divide, writing lhsT tiles for the output projection.
  - Output projection vs WoT [256, 1024], fp32 copy-back, DMA out.
"""

import numpy as np
from contextlib import ExitStack

import concourse.bass as bass
import concourse.tile as tile
from concourse import bacc, mybir
from concourse.bass_utils import run_bass_kernel_spmd

F32 = mybir.dt.float32
F32R = mybir.dt.float32r
AOP = mybir.AluOpType
AF = mybir.ActivationFunctionType

S = 2048          # seq len
DM = 1024         # model dim
HPC = 4           # heads per core
DH = 64           # head dim
CH = HPC * DH     # channels per core = 256
NCH = 4           # seq chunks (of 512) for projections / q-blocks
QB = S // NCH     # 512
KT = 128          # k tile
NKT = S // KT     # 16
ROPE_PERIOD = 10000.0


def _rope_tables():
    inv_freq = 1.0 / (ROPE_PERIOD ** (np.arange(0, DH, 2, dtype=np.float64) / DH))
    t = np.arange(S, dtype=np.float64)
    freqs = np.outer(inv_freq, t)           # [32, S]
    cos32 = np.cos(freqs).astype(np.float32)
    sin32 = np.sin(freqs).astype(np.float32)
    # cosT rows d: cos(s * invf[d % 32]), duplicated across the two
    # 64-row head slots of a head-pair tile.
    cos64 = np.concatenate([cos32, cos32], axis=0)          # [64, S]
    cosT = np.concatenate([cos64, cos64], axis=0)           # [128, S]
    # sin_signed2[d] multiplies q[d] and lands (after the 32-block swap)
    # on row d^32: rows 0:32 -> +sin (feeds out[32:64]), rows 32:64 -> -sin
    # (feeds out[0:32]).
    sin64 = np.concatenate([sin32, -sin32], axis=0)         # [64, S]
    sinT2 = np.concatenate([sin64, sin64], axis=0)          # [128, S]
    return cosT, sinT2


def _build():
    nc = bacc.Bacc(None, target_bir_lowering=False)

    xT_ext = nc.dram_tensor("xT", [DM, S], F32R, kind="ExternalInput")
    wqT_ext = nc.dram_tensor("wqT", [DM, CH], F32R, kind="ExternalInput")
    wkT_ext = nc.dram_tensor("wkT", [DM, CH], F32R, kind="ExternalInput")
    wvT_ext = nc.dram_tensor("wvT", [DM, CH], F32R, kind="ExternalInput")
    woT_ext = nc.dram_tensor("woT", [CH, DM], F32R, kind="ExternalInput")
    out_ext = nc.dram_tensor("out", [S, DM], F32, kind="ExternalOutput")

    cosT_np, sinT2_np = _rope_tables()
    cosT_dram = nc.inline_tensor(cosT_np, name="cosT")
    sinT2_dram = nc.inline_tensor(sinT2_np, name="sinT2")
    tri_np = np.where(np.arange(KT)[:, None] <= np.arange(KT)[None, :],
                      0.0, -1e9).astype(np.float32)
    tri_dram = nc.inline_tensor(tri_np, name="tri")

    with tile.TileContext(nc) as tc, ExitStack() as ctx:
        const = ctx.enter_context(tc.tile_pool(name="const", bufs=1))
        persist = ctx.enter_context(tc.tile_pool(name="persist", bufs=1))

        cosT = const.tile([128, S], F32, tag="cosT")
        sinT2 = const.tile([128, S], F32, tag="sinT2")
        tri = const.tile([KT, KT], F32, tag="tri")
        nc.sync.dma_start(cosT[:], cosT_dram[:])
        nc.sync.dma_start(sinT2[:], sinT2_dram[:])
        nc.sync.dma_start(tri[:], tri_dram[:])

        wq_t = [const.tile([128, CH], F32R, name=f"wq{k}") for k in range(8)]
        wk_t = [const.tile([128, CH], F32R, name=f"wk{k}") for k in range(8)]
        wv_t = [const.tile([128, CH], F32R, name=f"wv{k}") for k in range(8)]
        wo_t = [const.tile([128, DM], F32R, name=f"wo{k}") for k in range(2)]
        for k in range(8):
            nc.sync.dma_start(wq_t[k][:], wqT_ext[128 * k:128 * (k + 1), :])
            nc.sync.dma_start(wk_t[k][:], wkT_ext[128 * k:128 * (k + 1), :])
            nc.sync.dma_start(wv_t[k][:], wvT_ext[128 * k:128 * (k + 1), :])
        for k in range(2):
            nc.sync.dma_start(wo_t[k][:], woT_ext[128 * k:128 * (k + 1), :])

        # persistent activations (transposed layouts, head-pair tiles)
        qT_sb = [persist.tile([128, S], F32R, name=f"qT{m}") for m in range(2)]
        kT_sb = [persist.tile([128, S], F32R, name=f"kT{m}") for m in range(2)]
        attn_sb = [persist.tile([128, S], F32R, name=f"at{m}") for m in range(2)]
        v_sb = [persist.tile([128, HPC * (DH + 1)], F32R, name=f"v{k}")
                for k in range(NKT)]

        # ---------------- phase 1: QKV projections + RoPE ----------------
        with ExitStack() as pctx:
            xpool = pctx.enter_context(tc.tile_pool(name="xpool", bufs=4))
            ppool = pctx.enter_context(
                tc.tile_pool(name="ppool", bufs=1, space="PSUM"))
            rpool = pctx.enter_context(tc.tile_pool(name="rpool", bufs=3))

            for cn in range(NCH):
                cs = slice(QB * cn, QB * (cn + 1))
                q_ps = [ppool.tile([128, QB], F32, name=f"qp{m}", tag=f"qp{m}") for m in range(2)]
                k_ps = [ppool.tile([128, QB], F32, name=f"kp{m}", tag=f"kp{m}") for m in range(2)]
                v_ps = [ppool.tile([128, CH], F32, name=f"vp{sq}", tag=f"vp{sq}") for sq in range(4)]
                for kt in range(8):
                    xt = xpool.tile([128, QB], F32R, tag="xt")
                    nc.sync.dma_start(xt[:], xT_ext[128 * kt:128 * (kt + 1), cs])
                    st, sp = kt == 0, kt == 7
                    for m in range(2):
                        nc.tensor.matmul(q_ps[m][:], wq_t[kt][:, 128 * m:128 * (m + 1)],
                                         xt[:], start=st, stop=sp)
                        nc.tensor.matmul(k_ps[m][:], wk_t[kt][:, 128 * m:128 * (m + 1)],
                                         xt[:], start=st, stop=sp)
                    for sq in range(4):
                        nc.tensor.matmul(v_ps[sq][:], xt[:, 128 * sq:128 * (sq + 1)],
                                         wv_t[kt][:], start=st, stop=sp)

                # RoPE eviction for q and k head-pair tiles
                for ps_list, dst in ((q_ps, qT_sb), (k_ps, kT_sb)):
                    for m in range(2):
                        t_t = rpool.tile([128, QB], F32, tag="ropet")
                        r_t = rpool.tile([128, QB], F32, tag="roper")
                        rs_t = rpool.tile([128, QB], F32, tag="ropes")
                        nc.vector.tensor_tensor(
                            t_t[:], ps_list[m][:], cosT[:, cs], AOP.mult)
                        nc.vector.tensor_tensor(
                            r_t[:], ps_list[m][:], sinT2[:, cs], AOP.mult)
                        for blk, eng in zip(range(4), (nc.sync, nc.gpsimd,
                                                       nc.gpsimd, nc.sync)):
                            src = slice(32 * (blk ^ 1), 32 * (blk ^ 1) + 32)
                            dst_sl = slice(32 * blk, 32 * blk + 32)
                            eng.dma_start(rs_t[dst_sl, :], r_t[src, :])
                        nc.vector.tensor_tensor(
                            dst[m][:, cs], t_t[:], rs_t[:], AOP.add)

                # V eviction: interleave 4 heads with a ones column each
                for sq in range(4):
                    vt = v_sb[4 * cn + sq]
                    nc.vector.memset(vt[:].bitcast(F32), 1.0)
                    nc.vector.tensor_copy(
                        vt[:].rearrange("p (h e) -> p h e", h=HPC)[:, :, 0:64],
                        v_ps[sq][:].rearrange("p (h d) -> p h d", h=HPC))

        # ---------------- phase 2: attention + out-projection ----------------
        with ExitStack() as actx:
            scpool = actx.enter_context(
                tc.tile_pool(name="scpool", bufs=2, space="PSUM"))
            popool = actx.enter_context(
                tc.tile_pool(name="popool", bufs=1, space="PSUM"))
            ptpool = actx.enter_context(tc.tile_pool(name="ptpool", bufs=4))
            nrmpool = actx.enter_context(tc.tile_pool(name="nrmpool", bufs=2))
            opool = actx.enter_context(
                tc.tile_pool(name="opool", bufs=2, space="PSUM"))
            ospool = actx.enter_context(tc.tile_pool(name="ospool", bufs=3))

            for b in range(NCH):
                qs = slice(QB * b, QB * (b + 1))
                nkt = 4 * b + 4
                for m in range(2):
                    po = [popool.tile([DH + 1, QB], F32, name=f"po{hh}_{m}_{b}",
                                      tag=f"po{hh}") for hh in range(2)]
                    for ki in range(nkt):
                        d = ki - 4 * b
                        qlo = max(0, 128 * d)
                        sc = []
                        for hh in range(2):
                            hrow = slice(64 * hh, 64 * hh + 64)
                            s_t = scpool.tile([128, QB], F32,
                                              name=f"sc{hh}_{m}_{b}_{ki}",
                                              tag=f"sc{hh}")
                            nc.tensor.matmul(
                                s_t[:, qlo:QB],
                                kT_sb[m][hrow, 128 * ki:128 * (ki + 1)],
                                qT_sb[m][hrow, QB * b + qlo:QB * (b + 1)],
                                start=True, stop=True)
                            sc.append(s_t)
                        pt = []
                        for hh in range(2):
                            if d >= 0:
                                nc.vector.tensor_tensor(
                                    sc[hh][:, qlo:qlo + 128],
                                    sc[hh][:, qlo:qlo + 128], tri[:], AOP.add)
                            p_t = ptpool.tile([128, QB], F32R,
                                              name=f"pt{hh}_{m}_{b}_{ki}",
                                              tag=f"pt{hh}")
                            nc.scalar.activation(p_t[:, qlo:QB], sc[hh][:, qlo:QB],
                                                 AF.Exp, scale=0.125)
                            pt.append(p_t)
                        for hh in range(2):
                            h = 2 * m + hh
                            nc.tensor.matmul(
                                po[hh][:, qlo:QB],
                                v_sb[ki][:, 65 * h:65 * (h + 1)],
                                pt[hh][:, qlo:QB],
                                start=(ki == 0), stop=(ki == nkt - 1))
                    # normalize by the accumulated denominator row
                    for hh in range(2):
                        hrow = slice(64 * hh, 64 * hh + 64)
                        rsum = nrmpool.tile([1, QB], F32, tag=f"rsum{hh}")
                        rbc = nrmpool.tile([DH, QB], F32, tag=f"rbc{hh}")
                        nc.vector.reciprocal(rsum[:], po[hh][DH:DH + 1, :])
                        nc.gpsimd.partition_broadcast(rbc[:], rsum[:])
                        nc.vector.tensor_tensor(
                            attn_sb[m][hrow, qs], po[hh][0:DH, :], rbc[:],
                            AOP.mult)

                # out-projection for the 4 seq tiles of this block
                for sq in range(4 * b, 4 * b + 4):
                    ssl = slice(128 * sq, 128 * (sq + 1))
                    ot = ospool.tile([128, DM], F32, tag="ot")
                    for on in range(2):
                        osl = slice(512 * on, 512 * (on + 1))
                        ops = opool.tile([128, 512], F32, tag="ops")
                        for ct in range(2):
                            nc.tensor.matmul(ops[:], attn_sb[ct][:, ssl],
                                             wo_t[ct][:, osl],
                                             start=(ct == 0), stop=(ct == 1))
                        nc.scalar.copy(ot[:, osl], ops[:])
                    nc.gpsimd.dma_start(out_ext[ssl, :], ot[:])

    nc.compile()
    return nc


_NC_CACHE = []


def kernel(x, Wq, Wk, Wv, Wo):
    x = np.asarray(x, dtype=np.float32)
    Wq = np.asarray(Wq, dtype=np.float32)
    Wk = np.asarray(Wk, dtype=np.float32)
    Wv = np.asarray(Wv, dtype=np.float32)
    Wo = np.asarray(Wo, dtype=np.float32)

    in_maps = []
    for c in range(8):
        b, hg = divmod(c, 4)
        rows = slice(CH * hg, CH * (hg + 1))
        in_maps.append({
            "xT": np.ascontiguousarray(x[b].T),
            "wqT": np.ascontiguousarray(Wq[rows, :].T),
            "wkT": np.ascontiguousarray(Wk[rows, :].T),
            "wvT": np.ascontiguousarray(Wv[rows, :].T),
            "woT": np.ascontiguousarray(Wo[:, rows].T),
        })

    if not _NC_CACHE:
        _NC_CACHE.append(_build())
    nc = _NC_CACHE[0]

    res = run_bass_kernel_spmd(nc, in_maps, list(range(8)))
    out = np.zeros((2, S, DM), dtype=np.float32)
    for c in range(8):
        out[c // 4] += res.results[c]["out"]
    return out


# revision 11
# speedup vs baseline: 1.4611x; 1.1575x over previous
"""Causal multi-head attention mixer on 8 TRN2 NeuronCores.

Problem: x[2, 2048, 1024], 16 heads x 64 dim, RoPE, causal softmax, Wo proj.

Sharding (host-side): core c -> (batch b = c//4, head-group hg = c%4 of 4
heads = 256 channels). Each core computes its 4 heads' attention for its
batch and a partial output projection over its 256 Wo columns; the host
sums the 4 partials per batch. No on-device collectives.

Device kernel (per core), all matmuls in float32r (full PE rate, ~1.5e-4
rounding):
  - QKV projections from xT [1024, 2048] streamed in seq-chunks of 512,
    accumulating over the 8 K-tiles in PSUM.
  - RoPE applied on PSUM eviction of q/k (transposed layout [dim, seq]):
    out = q*cos + swap32(q*sin_signed), the 32-row block swap done with
    SBUF->SBUF DMAs.
  - Attention in transposed layout: scores.T [k-tile 128, q-block 512] =
    kT.T @ qT on PE; exp on ACT (scale=1/8) straight PSUM->SBUF (f32r);
    causal mask = one [128,128] triangular multiply on diagonal subtiles;
    P.T @ V via PE with a ones-column appended to V so the softmax
    denominators accumulate for free as row 64 of the output.
  - Normalize: gpsimd partition-broadcast of the denominator row, DVE
    divide, writing lhsT tiles for the output projection.
  - Output projection vs WoT [256, 1024], fp32 copy-back, DMA out.
"""

import numpy as np
import ml_dtypes
from contextlib import ExitStack

import concourse.bass as bass
import concourse.tile as tile
from concourse import bacc, mybir
from concourse.bass_utils import run_bass_kernel_spmd

import os

F32 = mybir.dt.float32
F32R = mybir.dt.float32r
BF16 = mybir.dt.bfloat16
CDT = BF16 if os.environ.get("BASS_CDT", "bf16") == "bf16" else F32R
AOP = mybir.AluOpType
AF = mybir.ActivationFunctionType

S = 2048          # seq len
DM = 1024         # model dim
HPC = 4           # heads per core
DH = 64           # head dim
CH = HPC * DH     # channels per core = 256
NCH = 4           # seq chunks (of 512) for projections / q-blocks
QB = S // NCH     # 512
KT = 128          # k tile
NKT = S // KT     # 16
ROPE_PERIOD = 10000.0


def _rope_tables():
    inv_freq = 1.0 / (ROPE_PERIOD ** (np.arange(0, DH, 2, dtype=np.float64) / DH))
    t = np.arange(S, dtype=np.float64)
    freqs = np.outer(inv_freq, t)           # [32, S]
    cos32 = np.cos(freqs).astype(np.float32)
    sin32 = np.sin(freqs).astype(np.float32)
    # cosT rows d: cos(s * invf[d % 32]), duplicated across the two
    # 64-row head slots of a head-pair tile.
    cos64 = np.concatenate([cos32, cos32], axis=0)          # [64, S]
    cosT = np.concatenate([cos64, cos64], axis=0)           # [128, S]
    # sin_signed2[d] multiplies q[d] and lands (after the 32-block swap)
    # on row d^32: rows 0:32 -> +sin (feeds out[32:64]), rows 32:64 -> -sin
    # (feeds out[0:32]).
    sin64 = np.concatenate([sin32, -sin32], axis=0)         # [64, S]
    sinT2 = np.concatenate([sin64, sin64], axis=0)          # [128, S]
    return cosT, sinT2


def _build():
    nc = bacc.Bacc(None, target_bir_lowering=False)

    xT_ext = nc.dram_tensor("xT", [DM, S], CDT, kind="ExternalInput")
    wqT_ext = nc.dram_tensor("wqT", [DM, CH], CDT, kind="ExternalInput")
    wkT_ext = nc.dram_tensor("wkT", [DM, CH], CDT, kind="ExternalInput")
    wvT_ext = nc.dram_tensor("wvT", [DM, CH], CDT, kind="ExternalInput")
    woT_ext = nc.dram_tensor("woT", [CH, DM], CDT, kind="ExternalInput")
    out_ext = nc.dram_tensor("out", [S, DM], F32, kind="ExternalOutput")

    cosT_np, sinT2_np = _rope_tables()
    cosT_dram = nc.inline_tensor(cosT_np, name="cosT")
    sinT2_dram = nc.inline_tensor(sinT2_np, name="sinT2")
    tri_np = np.where(np.arange(KT)[:, None] <= np.arange(KT)[None, :],
                      0.0, -1e9).astype(np.float32)
    tri_dram = nc.inline_tensor(tri_np, name="tri")

    with tile.TileContext(nc) as tc, ExitStack() as ctx:
        const = ctx.enter_context(tc.tile_pool(name="const", bufs=1))
        persist = ctx.enter_context(tc.tile_pool(name="persist", bufs=1))

        cosT = const.tile([128, S], F32, tag="cosT")
        sinT2 = const.tile([128, S], F32, tag="sinT2")
        tri = const.tile([KT, KT], F32, tag="tri")
        nc.sync.dma_start(cosT[:], cosT_dram[:])
        nc.sync.dma_start(sinT2[:], sinT2_dram[:])
        nc.sync.dma_start(tri[:], tri_dram[:])

        wq_t = [const.tile([128, CH], CDT, name=f"wq{k}") for k in range(8)]
        wk_t = [const.tile([128, CH], CDT, name=f"wk{k}") for k in range(8)]
        wv_t = [const.tile([128, CH], CDT, name=f"wv{k}") for k in range(8)]
        wo_t = [const.tile([128, DM], CDT, name=f"wo{k}") for k in range(2)]
        for k in range(8):
            nc.sync.dma_start(wq_t[k][:], wqT_ext[128 * k:128 * (k + 1), :])
            nc.sync.dma_start(wk_t[k][:], wkT_ext[128 * k:128 * (k + 1), :])
            nc.sync.dma_start(wv_t[k][:], wvT_ext[128 * k:128 * (k + 1), :])
        for k in range(2):
            nc.sync.dma_start(wo_t[k][:], woT_ext[128 * k:128 * (k + 1), :])

        # persistent activations (transposed layouts, head-pair tiles)
        qT_sb = [persist.tile([128, S], CDT, name=f"qT{m}") for m in range(2)]
        kT_sb = [persist.tile([128, S], CDT, name=f"kT{m}") for m in range(2)]
        attn_sb = [persist.tile([128, S], CDT, name=f"at{m}") for m in range(2)]
        v_sb = [persist.tile([128, HPC * (DH + 1)], CDT, name=f"v{k}")
                for k in range(NKT)]

        # ---------------- phase 1: QKV projections + RoPE ----------------
        with ExitStack() as pctx:
            xpool = pctx.enter_context(tc.tile_pool(name="xpool", bufs=4))
            ppool = pctx.enter_context(
                tc.tile_pool(name="ppool", bufs=1, space="PSUM"))
            rpool = pctx.enter_context(tc.tile_pool(name="rpool", bufs=3))

            for cn in range(NCH):
                cs = slice(QB * cn, QB * (cn + 1))
                q_ps = [ppool.tile([128, QB], F32, name=f"qp{m}", tag=f"qp{m}") for m in range(2)]
                k_ps = [ppool.tile([128, QB], F32, name=f"kp{m}", tag=f"kp{m}") for m in range(2)]
                v_ps = [ppool.tile([128, CH], F32, name=f"vp{sq}", tag=f"vp{sq}") for sq in range(4)]
                for kt in range(8):
                    xt = xpool.tile([128, QB], CDT, tag="xt")
                    nc.sync.dma_start(xt[:], xT_ext[128 * kt:128 * (kt + 1), cs])
                    st, sp = kt == 0, kt == 7
                    for m in range(2):
                        nc.tensor.matmul(q_ps[m][:], wq_t[kt][:, 128 * m:128 * (m + 1)],
                                         xt[:], start=st, stop=sp)
                        nc.tensor.matmul(k_ps[m][:], wk_t[kt][:, 128 * m:128 * (m + 1)],
                                         xt[:], start=st, stop=sp)
                    for sq in range(4):
                        nc.tensor.matmul(v_ps[sq][:], xt[:, 128 * sq:128 * (sq + 1)],
                                         wv_t[kt][:], start=st, stop=sp)

                # RoPE eviction for q and k head-pair tiles
                for ps_list, dst in ((q_ps, qT_sb), (k_ps, kT_sb)):
                    for m in range(2):
                        t_t = rpool.tile([128, QB], F32, tag="ropet")
                        r_t = rpool.tile([128, QB], F32, tag="roper")
                        rs_t = rpool.tile([128, QB], F32, tag="ropes")
                        nc.vector.tensor_tensor(
                            t_t[:], ps_list[m][:], cosT[:, cs], AOP.mult)
                        nc.vector.tensor_tensor(
                            r_t[:], ps_list[m][:], sinT2[:, cs], AOP.mult)
                        for blk, eng in zip(range(4), (nc.sync, nc.gpsimd,
                                                       nc.gpsimd, nc.sync)):
                            src = slice(32 * (blk ^ 1), 32 * (blk ^ 1) + 32)
                            dst_sl = slice(32 * blk, 32 * blk + 32)
                            eng.dma_start(rs_t[dst_sl, :], r_t[src, :])
                        nc.vector.tensor_tensor(
                            dst[m][:, cs], t_t[:], rs_t[:], AOP.add)

                # V eviction: interleave 4 heads with a ones column each
                for sq in range(4):
                    vt = v_sb[4 * cn + sq]
                    nc.vector.memset(vt[:] if CDT == BF16 else vt[:].bitcast(F32), 1.0)
                    nc.vector.tensor_copy(
                        vt[:].rearrange("p (h e) -> p h e", h=HPC)[:, :, 0:64],
                        v_ps[sq][:].rearrange("p (h d) -> p h d", h=HPC))

        # ---------------- phase 2: attention + out-projection ----------------
        with ExitStack() as actx:
            scpool = actx.enter_context(
                tc.tile_pool(name="scpool", bufs=2, space="PSUM"))
            popool = actx.enter_context(
                tc.tile_pool(name="popool", bufs=1, space="PSUM"))
            ptpool = actx.enter_context(tc.tile_pool(name="ptpool", bufs=4))
            nrmpool = actx.enter_context(tc.tile_pool(name="nrmpool", bufs=2))
            opool = actx.enter_context(
                tc.tile_pool(name="opool", bufs=2, space="PSUM"))
            ospool = actx.enter_context(tc.tile_pool(name="ospool", bufs=3))

            for b in range(NCH):
                qs = slice(QB * b, QB * (b + 1))
                nkt = 4 * b + 4
                for m in range(2):
                    po = [popool.tile([DH + 1, QB], F32, name=f"po{hh}_{m}_{b}",
                                      tag=f"po{hh}") for hh in range(2)]
                    for ki in range(nkt):
                        d = ki - 4 * b
                        qlo = max(0, 128 * d)
                        sc = []
                        for hh in range(2):
                            hrow = slice(64 * hh, 64 * hh + 64)
                            s_t = scpool.tile([128, QB], F32,
                                              name=f"sc{hh}_{m}_{b}_{ki}",
                                              tag=f"sc{hh}")
                            nc.tensor.matmul(
                                s_t[:, qlo:QB],
                                kT_sb[m][hrow, 128 * ki:128 * (ki + 1)],
                                qT_sb[m][hrow, QB * b + qlo:QB * (b + 1)],
                                start=True, stop=True)
                            sc.append(s_t)
                        pt = []
                        for hh in range(2):
                            if d >= 0:
                                nc.vector.tensor_tensor(
                                    sc[hh][:, qlo:qlo + 128],
                                    sc[hh][:, qlo:qlo + 128], tri[:], AOP.add)
                            p_t = ptpool.tile([128, QB], CDT,
                                              name=f"pt{hh}_{m}_{b}_{ki}",
                                              tag=f"pt{hh}")
                            nc.scalar.activation(p_t[:, qlo:QB], sc[hh][:, qlo:QB],
                                                 AF.Exp, scale=0.125)
                            pt.append(p_t)
                        for hh in range(2):
                            h = 2 * m + hh
                            nc.tensor.matmul(
                                po[hh][:, qlo:QB],
                                v_sb[ki][:, 65 * h:65 * (h + 1)],
                                pt[hh][:, qlo:QB],
                                start=(ki == 0), stop=(ki == nkt - 1))
                    # normalize by the accumulated denominator row
                    for hh in range(2):
                        hrow = slice(64 * hh, 64 * hh + 64)
                        rsum = nrmpool.tile([1, QB], F32, tag=f"rsum{hh}")
                        rbc = nrmpool.tile([DH, QB], F32, tag=f"rbc{hh}")
                        nc.vector.reciprocal(rsum[:], po[hh][DH:DH + 1, :])
                        nc.gpsimd.partition_broadcast(rbc[:], rsum[:])
                        nc.vector.tensor_tensor(
                            attn_sb[m][hrow, qs], po[hh][0:DH, :], rbc[:],
                            AOP.mult)

                # out-projection for the 4 seq tiles of this block
                for sq in range(4 * b, 4 * b + 4):
                    ssl = slice(128 * sq, 128 * (sq + 1))
                    ot = ospool.tile([128, DM], F32, tag="ot")
                    for on in range(2):
                        osl = slice(512 * on, 512 * (on + 1))
                        ops = opool.tile([128, 512], F32, tag="ops")
                        for ct in range(2):
                            nc.tensor.matmul(ops[:], attn_sb[ct][:, ssl],
                                             wo_t[ct][:, osl],
                                             start=(ct == 0), stop=(ct == 1))
                        nc.scalar.copy(ot[:, osl], ops[:])
                    nc.gpsimd.dma_start(out_ext[ssl, :], ot[:])

    nc.compile()
    return nc


_NC_CACHE = []


def kernel(x, Wq, Wk, Wv, Wo):
    x = np.asarray(x, dtype=np.float32)
    Wq = np.asarray(Wq, dtype=np.float32)
    Wk = np.asarray(Wk, dtype=np.float32)
    Wv = np.asarray(Wv, dtype=np.float32)
    Wo = np.asarray(Wo, dtype=np.float32)

    np_cdt = ml_dtypes.bfloat16 if CDT == BF16 else np.float32
    in_maps = []
    for c in range(8):
        b, hg = divmod(c, 4)
        rows = slice(CH * hg, CH * (hg + 1))
        in_maps.append({
            "xT": np.ascontiguousarray(x[b].T).astype(np_cdt),
            "wqT": np.ascontiguousarray(Wq[rows, :].T).astype(np_cdt),
            "wkT": np.ascontiguousarray(Wk[rows, :].T).astype(np_cdt),
            "wvT": np.ascontiguousarray(Wv[rows, :].T).astype(np_cdt),
            "woT": np.ascontiguousarray(Wo[:, rows].T).astype(np_cdt),
        })

    if not _NC_CACHE:
        _NC_CACHE.append(_build())
    nc = _NC_CACHE[0]

    res = run_bass_kernel_spmd(nc, in_maps, list(range(8)))
    out = np.zeros((2, S, DM), dtype=np.float32)
    for c in range(8):
        out[c // 4] += res.results[c]["out"]
    return out


# revision 13
# speedup vs baseline: 1.6106x; 1.1023x over previous
"""Causal multi-head attention mixer on 8 TRN2 NeuronCores.

Problem: x[2, 2048, 1024], 16 heads x 64 dim, RoPE, causal softmax, Wo proj.

Sharding (host-side): core c -> (batch b = c//4, head-group hg = c%4 of 4
heads = 256 channels). Each core computes its 4 heads' attention for its
batch and a partial output projection over its 256 Wo columns; the host
sums the 4 partials per batch. No on-device collectives.

Device kernel (per core), all matmuls in float32r (full PE rate, ~1.5e-4
rounding):
  - QKV projections from xT [1024, 2048] streamed in seq-chunks of 512,
    accumulating over the 8 K-tiles in PSUM.
  - RoPE applied on PSUM eviction of q/k (transposed layout [dim, seq]):
    out = q*cos + swap32(q*sin_signed), the 32-row block swap done with
    SBUF->SBUF DMAs.
  - Attention in transposed layout: scores.T [k-tile 128, q-block 512] =
    kT.T @ qT on PE; exp on ACT (scale=1/8) straight PSUM->SBUF (f32r);
    causal mask = one [128,128] triangular multiply on diagonal subtiles;
    P.T @ V via PE with a ones-column appended to V so the softmax
    denominators accumulate for free as row 64 of the output.
  - Normalize: gpsimd partition-broadcast of the denominator row, DVE
    divide, writing lhsT tiles for the output projection.
  - Output projection vs WoT [256, 1024], fp32 copy-back, DMA out.
"""

import numpy as np
import ml_dtypes
from contextlib import ExitStack

import concourse.bass as bass
import concourse.tile as tile
from concourse import bacc, mybir
from concourse.bass_utils import run_bass_kernel_spmd

import os

F32 = mybir.dt.float32
F32R = mybir.dt.float32r
BF16 = mybir.dt.bfloat16
CDT = BF16 if os.environ.get("BASS_CDT", "bf16") == "bf16" else F32R
AOP = mybir.AluOpType
AF = mybir.ActivationFunctionType

S = 2048          # seq len
DM = 1024         # model dim
HPC = 4           # heads per core
DH = 64           # head dim
CH = HPC * DH     # channels per core = 256
NCH = 4           # seq chunks (of 512) for projections / q-blocks
QB = S // NCH     # 512
KT = 128          # k tile
NKT = S // KT     # 16
ROPE_PERIOD = 10000.0


def _rope_tables():
    inv_freq = 1.0 / (ROPE_PERIOD ** (np.arange(0, DH, 2, dtype=np.float64) / DH))
    t = np.arange(S, dtype=np.float64)
    freqs = np.outer(inv_freq, t)           # [32, S]
    cos32 = np.cos(freqs).astype(np.float32)
    sin32 = np.sin(freqs).astype(np.float32)
    # cosT rows d: cos(s * invf[d % 32]), duplicated across the two
    # 64-row head slots of a head-pair tile.
    cos64 = np.concatenate([cos32, cos32], axis=0)          # [64, S]
    cosT = np.concatenate([cos64, cos64], axis=0)           # [128, S]
    # sin_signed2[d] multiplies q[d] and lands (after the 32-block swap)
    # on row d^32: rows 0:32 -> +sin (feeds out[32:64]), rows 32:64 -> -sin
    # (feeds out[0:32]).
    sin64 = np.concatenate([sin32, -sin32], axis=0)         # [64, S]
    sinT2 = np.concatenate([sin64, sin64], axis=0)          # [128, S]
    return cosT, sinT2


def _build():
    nc = bacc.Bacc(None, target_bir_lowering=False)

    xT_ext = nc.dram_tensor("xT", [DM, S], CDT, kind="ExternalInput")
    wqT_ext = nc.dram_tensor("wqT", [DM, CH], CDT, kind="ExternalInput")
    wkT_ext = nc.dram_tensor("wkT", [DM, CH], CDT, kind="ExternalInput")
    wvT_ext = nc.dram_tensor("wvT", [DM, CH], CDT, kind="ExternalInput")
    woT_ext = nc.dram_tensor("woT", [CH, DM], CDT, kind="ExternalInput")
    out_ext = nc.dram_tensor("out", [S, DM], F32, kind="ExternalOutput")

    cosT_np, sinT2_np = _rope_tables()
    cosT_dram = nc.inline_tensor(cosT_np, name="cosT")
    sinT2_dram = nc.inline_tensor(sinT2_np, name="sinT2")
    tri_np = np.where(np.arange(KT)[:, None] <= np.arange(KT)[None, :],
                      0.0, -1e9).astype(np.float32)
    tri_dram = nc.inline_tensor(tri_np, name="tri")
    ones_np = np.ones((128, HPC), dtype=ml_dtypes.bfloat16)
    ones_dram = nc.inline_tensor(ones_np, name="ones4")

    with tile.TileContext(nc) as tc, ExitStack() as ctx:
        const = ctx.enter_context(tc.tile_pool(name="const", bufs=1))
        persist = ctx.enter_context(tc.tile_pool(name="persist", bufs=1))

        cosT = const.tile([128, S], F32, tag="cosT")
        sinT2 = const.tile([128, S], F32, tag="sinT2")
        tri = const.tile([KT, KT], F32, tag="tri")
        ones4 = const.tile([128, HPC], CDT, tag="ones4")
        nc.sync.dma_start(cosT[:], cosT_dram[:])
        nc.sync.dma_start(sinT2[:], sinT2_dram[:])
        nc.sync.dma_start(tri[:], tri_dram[:])
        nc.sync.dma_start(ones4[:], ones_dram[:])

        wq_t = [const.tile([128, CH], CDT, name=f"wq{k}") for k in range(8)]
        wk_t = [const.tile([128, CH], CDT, name=f"wk{k}") for k in range(8)]
        wv_t = [const.tile([128, CH], CDT, name=f"wv{k}") for k in range(8)]
        wo_t = [const.tile([128, DM], CDT, name=f"wo{k}") for k in range(2)]
        for k in range(8):
            nc.sync.dma_start(wq_t[k][:], wqT_ext[128 * k:128 * (k + 1), :])
            nc.sync.dma_start(wk_t[k][:], wkT_ext[128 * k:128 * (k + 1), :])
            nc.sync.dma_start(wv_t[k][:], wvT_ext[128 * k:128 * (k + 1), :])
        for k in range(2):
            nc.sync.dma_start(wo_t[k][:], woT_ext[128 * k:128 * (k + 1), :])

        # persistent activations (transposed layouts, head-pair tiles)
        qT_sb = [persist.tile([128, S], CDT, name=f"qT{m}") for m in range(2)]
        kp_sb = [persist.tile([128, S], CDT, name=f"kp{h}") for h in range(HPC)]
        attn_sb = [persist.tile([128, S], CDT, name=f"at{m}") for m in range(2)]
        v_sb = [persist.tile([128, HPC * 128], CDT, name=f"v{k}")
                for k in range(NKT)]
        # zero the dead head-half of each kp tile once
        for h in range(HPC):
            dead = slice(0, 64) if h % 2 else slice(64, 128)
            nc.vector.memset(kp_sb[h][dead, :], 0.0)

        # ---------------- phase 1: QKV projections + RoPE ----------------
        with ExitStack() as pctx:
            xpool = pctx.enter_context(tc.tile_pool(name="xpool", bufs=4))
            ppool = pctx.enter_context(
                tc.tile_pool(name="ppool", bufs=1, space="PSUM"))
            rpool = pctx.enter_context(tc.tile_pool(name="rpool", bufs=3))

            for cn in range(NCH):
                cs = slice(QB * cn, QB * (cn + 1))
                q_ps = [ppool.tile([128, QB], F32, name=f"qp{m}", tag=f"qp{m}") for m in range(2)]
                k_ps = [ppool.tile([128, QB], F32, name=f"kp{m}", tag=f"kp{m}") for m in range(2)]
                v_ps = [ppool.tile([128, CH], F32, name=f"vp{sq}", tag=f"vp{sq}") for sq in range(4)]
                for kt in range(8):
                    xt = xpool.tile([128, QB], CDT, tag="xt")
                    nc.sync.dma_start(xt[:], xT_ext[128 * kt:128 * (kt + 1), cs])
                    st, sp = kt == 0, kt == 7
                    for m in range(2):
                        nc.tensor.matmul(q_ps[m][:], wq_t[kt][:, 128 * m:128 * (m + 1)],
                                         xt[:], start=st, stop=sp)
                        nc.tensor.matmul(k_ps[m][:], wk_t[kt][:, 128 * m:128 * (m + 1)],
                                         xt[:], start=st, stop=sp)
                    for sq in range(4):
                        nc.tensor.matmul(v_ps[sq][:], xt[:, 128 * sq:128 * (sq + 1)],
                                         wv_t[kt][:], start=st, stop=sp)

                # RoPE eviction: q into pair tiles, k into padded per-head
                for ps_list, is_q in ((q_ps, True), (k_ps, False)):
                    for m in range(2):
                        t_t = rpool.tile([128, QB], F32, tag="ropet")
                        r_t = rpool.tile([128, QB], F32, tag="roper")
                        rs_t = rpool.tile([128, QB], F32, tag="ropes")
                        nc.vector.tensor_tensor(
                            t_t[:], ps_list[m][:], cosT[:, cs], AOP.mult)
                        nc.vector.tensor_tensor(
                            r_t[:], ps_list[m][:], sinT2[:, cs], AOP.mult)
                        for blk, eng in zip(range(4), (nc.sync, nc.gpsimd,
                                                       nc.gpsimd, nc.sync)):
                            src = slice(32 * (blk ^ 1), 32 * (blk ^ 1) + 32)
                            dst_sl = slice(32 * blk, 32 * blk + 32)
                            eng.dma_start(rs_t[dst_sl, :], r_t[src, :])
                        if is_q:
                            nc.vector.tensor_tensor(
                                qT_sb[m][:, cs], t_t[:], rs_t[:], AOP.add)
                        else:
                            for hh in range(2):
                                hrow = slice(64 * hh, 64 * hh + 64)
                                nc.vector.tensor_tensor(
                                    kp_sb[2 * m + hh][hrow, cs],
                                    t_t[hrow, :], rs_t[hrow, :], AOP.add)

                # V eviction: interleave 4 heads with a ones column each
                for sq in range(4):
                    vt = v_sb[4 * cn + sq]
                    nc.vector.memset(vt[:], 0.0)
                    vt3 = vt[:].rearrange("p (h e) -> p h e", h=HPC)
                    nc.vector.tensor_copy(
                        vt3[:, :, 0:64],
                        v_ps[sq][:].rearrange("p (h d) -> p h d", h=HPC))
                    nc.vector.tensor_copy(vt3[:, :, 64:65],
                                          ones4[:].unsqueeze(-1))

        # ---------------- phase 2: attention + out-projection ----------------
        with ExitStack() as actx:
            scpool = actx.enter_context(
                tc.tile_pool(name="scpool", bufs=2, space="PSUM"))
            popool = actx.enter_context(
                tc.tile_pool(name="popool", bufs=1, space="PSUM"))
            ptpool = actx.enter_context(tc.tile_pool(name="ptpool", bufs=4))
            nrmpool = actx.enter_context(tc.tile_pool(name="nrmpool", bufs=2))
            opool = actx.enter_context(
                tc.tile_pool(name="opool", bufs=2, space="PSUM"))
            ospool = actx.enter_context(tc.tile_pool(name="ospool", bufs=3))

            for b in range(NCH):
                qs = slice(QB * b, QB * (b + 1))
                nkt = 4 * b + 4
                for m in range(2):
                    po = [popool.tile([128, QB], F32, name=f"po{hh}_{m}_{b}",
                                      tag=f"po{hh}") for hh in range(2)]
                    for ki in range(nkt):
                        d = ki - 4 * b
                        qlo = max(0, 128 * d)
                        sc = []
                        for hh in range(2):
                            s_t = scpool.tile([128, QB], F32,
                                              name=f"sc{hh}_{m}_{b}_{ki}",
                                              tag=f"sc{hh}")
                            nc.tensor.matmul(
                                s_t[:, qlo:QB],
                                kp_sb[2 * m + hh][:, 128 * ki:128 * (ki + 1)],
                                qT_sb[m][:, QB * b + qlo:QB * (b + 1)],
                                start=True, stop=True)
                            sc.append(s_t)
                        pt = []
                        for hh in range(2):
                            if d >= 0:
                                nc.vector.tensor_tensor(
                                    sc[hh][:, qlo:qlo + 128],
                                    sc[hh][:, qlo:qlo + 128], tri[:], AOP.add)
                            p_t = ptpool.tile([128, QB], CDT,
                                              name=f"pt{hh}_{m}_{b}_{ki}",
                                              tag=f"pt{hh}")
                            nc.scalar.activation(p_t[:, qlo:QB], sc[hh][:, qlo:QB],
                                                 AF.Exp, scale=0.125)
                            pt.append(p_t)
                        for hh in range(2):
                            h = 2 * m + hh
                            nc.tensor.matmul(
                                po[hh][:, qlo:QB],
                                v_sb[ki][:, 128 * h:128 * (h + 1)],
                                pt[hh][:, qlo:QB],
                                start=(ki == 0), stop=(ki == nkt - 1))
                    # normalize by the accumulated denominator row
                    for hh in range(2):
                        hrow = slice(64 * hh, 64 * hh + 64)
                        lsum = nrmpool.tile([1, QB], F32, tag=f"lsum{hh}")
                        rsum = nrmpool.tile([1, QB], F32, tag=f"rsum{hh}")
                        rbc = nrmpool.tile([DH, QB], F32, tag=f"rbc{hh}")
                        nc.scalar.activation(lsum[:], po[hh][DH:DH + 1, :],
                                             AF.Ln)
                        nc.scalar.activation(rsum[:], lsum[:], AF.Exp,
                                             scale=-1.0)
                        nc.gpsimd.partition_broadcast(rbc[:], rsum[:])
                        nc.vector.tensor_tensor(
                            attn_sb[m][hrow, qs], po[hh][0:DH, :], rbc[:],
                            AOP.mult)

                # out-projection for the 4 seq tiles of this block
                for sq in range(4 * b, 4 * b + 4):
                    ssl = slice(128 * sq, 128 * (sq + 1))
                    ot = ospool.tile([128, DM], F32, tag="ot")
                    for on in range(2):
                        osl = slice(512 * on, 512 * (on + 1))
                        ops = opool.tile([128, 512], F32, tag="ops")
                        for ct in range(2):
                            nc.tensor.matmul(ops[:], attn_sb[ct][:, ssl],
                                             wo_t[ct][:, osl],
                                             start=(ct == 0), stop=(ct == 1))
                        nc.scalar.copy(ot[:, osl], ops[:])
                    nc.gpsimd.dma_start(out_ext[ssl, :], ot[:])

    nc.compile()
    return nc


_NC_CACHE = []


def kernel(x, Wq, Wk, Wv, Wo):
    x = np.asarray(x, dtype=np.float32)
    Wq = np.asarray(Wq, dtype=np.float32)
    Wk = np.asarray(Wk, dtype=np.float32)
    Wv = np.asarray(Wv, dtype=np.float32)
    Wo = np.asarray(Wo, dtype=np.float32)

    np_cdt = ml_dtypes.bfloat16 if CDT == BF16 else np.float32
    in_maps = []
    for c in range(8):
        b, hg = divmod(c, 4)
        rows = slice(CH * hg, CH * (hg + 1))
        in_maps.append({
            "xT": np.ascontiguousarray(x[b].T).astype(np_cdt),
            "wqT": np.ascontiguousarray(Wq[rows, :].T).astype(np_cdt),
            "wkT": np.ascontiguousarray(Wk[rows, :].T).astype(np_cdt),
            "wvT": np.ascontiguousarray(Wv[rows, :].T).astype(np_cdt),
            "woT": np.ascontiguousarray(Wo[:, rows].T).astype(np_cdt),
        })

    if not _NC_CACHE:
        _NC_CACHE.append(_build())
    nc = _NC_CACHE[0]

    res = run_bass_kernel_spmd(nc, in_maps, list(range(8)))
    out = np.zeros((2, S, DM), dtype=np.float32)
    for c in range(8):
        out[c // 4] += res.results[c]["out"]
    return out


# revision 15
# speedup vs baseline: 1.6348x; 1.0150x over previous
"""Causal multi-head attention mixer on 8 TRN2 NeuronCores.

Problem: x[2, 2048, 1024], 16 heads x 64 dim, RoPE, causal softmax, Wo proj.

Sharding (host-side): core c -> (batch b = c//4, head-group hg = c%4 of 4
heads = 256 channels). Each core computes its 4 heads' attention for its
batch and a partial output projection over its 256 Wo columns; the host
sums the 4 partials per batch. No on-device collectives.

Device kernel (per core), all matmuls in float32r (full PE rate, ~1.5e-4
rounding):
  - QKV projections from xT [1024, 2048] streamed in seq-chunks of 512,
    accumulating over the 8 K-tiles in PSUM.
  - RoPE applied on PSUM eviction of q/k (transposed layout [dim, seq]):
    out = q*cos + swap32(q*sin_signed), the 32-row block swap done with
    SBUF->SBUF DMAs.
  - Attention in transposed layout: scores.T [k-tile 128, q-block 512] =
    kT.T @ qT on PE; exp on ACT (scale=1/8) straight PSUM->SBUF (f32r);
    causal mask = one [128,128] triangular multiply on diagonal subtiles;
    P.T @ V via PE with a ones-column appended to V so the softmax
    denominators accumulate for free as row 64 of the output.
  - Normalize: gpsimd partition-broadcast of the denominator row, DVE
    divide, writing lhsT tiles for the output projection.
  - Output projection vs WoT [256, 1024], fp32 copy-back, DMA out.
"""

import numpy as np
import ml_dtypes
from contextlib import ExitStack

import concourse.bass as bass
import concourse.tile as tile
from concourse import bacc, mybir
from concourse.bass_utils import run_bass_kernel_spmd

import os

F32 = mybir.dt.float32
F32R = mybir.dt.float32r
BF16 = mybir.dt.bfloat16
CDT = BF16 if os.environ.get("BASS_CDT", "bf16") == "bf16" else F32R
AOP = mybir.AluOpType
AF = mybir.ActivationFunctionType

S = 2048          # seq len
DM = 1024         # model dim
HPC = 4           # heads per core
DH = 64           # head dim
CH = HPC * DH     # channels per core = 256
NCH = 4           # seq chunks (of 512) for projections / q-blocks
QB = S // NCH     # 512
KT = 128          # k tile
NKT = S // KT     # 16
ROPE_PERIOD = 10000.0


def _rope_tables():
    inv_freq = 1.0 / (ROPE_PERIOD ** (np.arange(0, DH, 2, dtype=np.float64) / DH))
    t = np.arange(S, dtype=np.float64)
    freqs = np.outer(inv_freq, t)           # [32, S]
    cos32 = np.cos(freqs).astype(np.float32)
    sin32 = np.sin(freqs).astype(np.float32)
    # cosT rows d: cos(s * invf[d % 32]), duplicated across the two
    # 64-row head slots of a head-pair tile.
    cos64 = np.concatenate([cos32, cos32], axis=0)          # [64, S]
    cosT = np.concatenate([cos64, cos64], axis=0)           # [128, S]
    # sin_signed2[d] multiplies q[d] and lands (after the 32-block swap)
    # on row d^32: rows 0:32 -> +sin (feeds out[32:64]), rows 32:64 -> -sin
    # (feeds out[0:32]).
    sin64 = np.concatenate([sin32, -sin32], axis=0)         # [64, S]
    sinT2 = np.concatenate([sin64, sin64], axis=0)          # [128, S]
    return cosT, sinT2


def _build():
    nc = bacc.Bacc(None, target_bir_lowering=False)

    xT_ext = nc.dram_tensor("xT", [DM, S], CDT, kind="ExternalInput")
    wqT_ext = nc.dram_tensor("wqT", [DM, CH], CDT, kind="ExternalInput")
    wkT_ext = nc.dram_tensor("wkT", [DM, CH], CDT, kind="ExternalInput")
    wvT_ext = nc.dram_tensor("wvT", [DM, CH], CDT, kind="ExternalInput")
    woT_ext = nc.dram_tensor("woT", [CH, DM], CDT, kind="ExternalInput")
    out_ext = nc.dram_tensor("out", [S, DM], F32, kind="ExternalOutput")

    cosT_np, sinT2_np = _rope_tables()
    # duplicate each 512-chunk so one wide op covers the [q|k] psum pair
    def _widen(t):
        return np.concatenate(
            [np.concatenate([t[:, 512 * c:512 * (c + 1)]] * 2, axis=1)
             for c in range(NCH)], axis=1)
    cosW_dram = nc.inline_tensor(_widen(cosT_np), name="cosW")
    sinW_dram = nc.inline_tensor(_widen(sinT2_np), name="sinW")
    tri_np = np.where(np.arange(KT)[:, None] <= np.arange(KT)[None, :],
                      0.0, -1e9).astype(np.float32)
    tri_dram = nc.inline_tensor(tri_np, name="tri")
    ones_np = np.ones((128, HPC), dtype=ml_dtypes.bfloat16)
    ones_dram = nc.inline_tensor(ones_np, name="ones4")

    with tile.TileContext(nc) as tc, ExitStack() as ctx:
        const = ctx.enter_context(tc.tile_pool(name="const", bufs=1))
        persist = ctx.enter_context(tc.tile_pool(name="persist", bufs=1))

        cosW = const.tile([128, 2 * S], F32, tag="cosW")
        sinW = const.tile([128, 2 * S], F32, tag="sinW")
        tri = const.tile([KT, KT], F32, tag="tri")
        ones4 = const.tile([128, HPC], CDT, tag="ones4")
        nc.sync.dma_start(cosW[:], cosW_dram[:])
        nc.sync.dma_start(sinW[:], sinW_dram[:])
        nc.sync.dma_start(tri[:], tri_dram[:])
        nc.sync.dma_start(ones4[:], ones_dram[:])

        wq_t = [const.tile([128, CH], CDT, name=f"wq{k}") for k in range(8)]
        wk_t = [const.tile([128, CH], CDT, name=f"wk{k}") for k in range(8)]
        wv_t = [const.tile([128, CH], CDT, name=f"wv{k}") for k in range(8)]
        wo_t = [const.tile([128, DM], CDT, name=f"wo{k}") for k in range(2)]
        for k in range(8):
            nc.sync.dma_start(wq_t[k][:], wqT_ext[128 * k:128 * (k + 1), :])
            nc.sync.dma_start(wk_t[k][:], wkT_ext[128 * k:128 * (k + 1), :])
            nc.sync.dma_start(wv_t[k][:], wvT_ext[128 * k:128 * (k + 1), :])
        for k in range(2):
            nc.sync.dma_start(wo_t[k][:], woT_ext[128 * k:128 * (k + 1), :])

        # persistent activations (transposed layouts, head-pair tiles)
        qT_sb = [persist.tile([128, S], CDT, name=f"qT{m}") for m in range(2)]
        kp_sb = [persist.tile([128, S], CDT, name=f"kp{h}") for h in range(HPC)]
        attn_sb = [persist.tile([128, S], CDT, name=f"at{m}") for m in range(2)]
        v_sb = [persist.tile([128, HPC * 128], CDT, name=f"v{k}")
                for k in range(NKT)]
        # zero the dead head-half of each kp tile once
        for h in range(HPC):
            dead = slice(0, 64) if h % 2 else slice(64, 128)
            nc.vector.memset(kp_sb[h][dead, :], 0.0)

        # ---------------- phase 1: QKV projections + RoPE ----------------
        with ExitStack() as pctx:
            xpool = pctx.enter_context(tc.tile_pool(name="xpool", bufs=4))
            ppool = pctx.enter_context(
                tc.tile_pool(name="ppool", bufs=1, space="PSUM"))
            rpool = pctx.enter_context(tc.tile_pool(name="rpool", bufs=3))

            for cn in range(NCH):
                cs = slice(QB * cn, QB * (cn + 1))
                qk_ps = [ppool.tile([128, 2 * QB], F32, name=f"qkp{m}_{cn}",
                                    tag="qkp", bufs=3) for m in range(2)]
                v_ps = [ppool.tile([128, 2 * CH], F32, name=f"vp{j}_{cn}",
                                   tag=f"vp{j}") for j in range(2)]
                for kt in range(8):
                    xt = xpool.tile([128, QB], CDT, tag="xt")
                    nc.sync.dma_start(xt[:], xT_ext[128 * kt:128 * (kt + 1), cs])
                    st, sp = kt == 0, kt == 7
                    for m in range(2):
                        nc.tensor.matmul(qk_ps[m][:, 0:QB],
                                         wq_t[kt][:, 128 * m:128 * (m + 1)],
                                         xt[:], start=st, stop=sp)
                        nc.tensor.matmul(qk_ps[m][:, QB:2 * QB],
                                         wk_t[kt][:, 128 * m:128 * (m + 1)],
                                         xt[:], start=st, stop=sp)
                    for sq in range(4):
                        nc.tensor.matmul(
                            v_ps[sq // 2][:, CH * (sq % 2):CH * (sq % 2 + 1)],
                            xt[:, 128 * sq:128 * (sq + 1)],
                            wv_t[kt][:],
                            start=(st and sq % 2 == 0),
                            stop=(sp and sq % 2 == 1))

                # RoPE eviction on the merged [q|k] psum pairs
                for m in range(2):
                    t_t = rpool.tile([128, 2 * QB], F32, tag="ropet")
                    r_t = rpool.tile([128, 2 * QB], F32, tag="roper")
                    rs_t = rpool.tile([128, 2 * QB], F32, tag="ropes")
                    nc.vector.tensor_tensor(
                        t_t[:], qk_ps[m][:], cosW[:, 2 * QB * cn:2 * QB * (cn + 1)],
                        AOP.mult)
                    nc.vector.tensor_tensor(
                        r_t[:], qk_ps[m][:], sinW[:, 2 * QB * cn:2 * QB * (cn + 1)],
                        AOP.mult)
                    for blk, eng in zip(range(4), (nc.sync, nc.gpsimd,
                                                   nc.gpsimd, nc.sync)):
                        src = slice(32 * (blk ^ 1), 32 * (blk ^ 1) + 32)
                        dst_sl = slice(32 * blk, 32 * blk + 32)
                        eng.dma_start(rs_t[dst_sl, :], r_t[src, :])
                    nc.vector.tensor_tensor(
                        qT_sb[m][:, cs], t_t[:, 0:QB], rs_t[:, 0:QB], AOP.add)
                    for hh in range(2):
                        hrow = slice(64 * hh, 64 * hh + 64)
                        nc.vector.tensor_tensor(
                            kp_sb[2 * m + hh][hrow, cs],
                            t_t[hrow, QB:2 * QB], rs_t[hrow, QB:2 * QB],
                            AOP.add)

                # V eviction: interleave 4 heads with a ones column each
                for sq in range(4):
                    vt = v_sb[4 * cn + sq]
                    nc.vector.memset(vt[:], 0.0)
                    vt3 = vt[:].rearrange("p (h e) -> p h e", h=HPC)
                    vsrc = v_ps[sq // 2][:, CH * (sq % 2):CH * (sq % 2 + 1)]
                    nc.vector.tensor_copy(
                        vt3[:, :, 0:64],
                        vsrc.rearrange("p (h d) -> p h d", h=HPC))
                    nc.vector.tensor_copy(vt3[:, :, 64:65],
                                          ones4[:].unsqueeze(-1))

        # ---------------- phase 2: attention + out-projection ----------------
        with ExitStack() as actx:
            scpool = actx.enter_context(
                tc.tile_pool(name="scpool", bufs=2, space="PSUM"))
            popool = actx.enter_context(
                tc.tile_pool(name="popool", bufs=1, space="PSUM"))
            ptpool = actx.enter_context(tc.tile_pool(name="ptpool", bufs=4))
            nrmpool = actx.enter_context(tc.tile_pool(name="nrmpool", bufs=2))
            opool = actx.enter_context(
                tc.tile_pool(name="opool", bufs=2, space="PSUM"))
            ospool = actx.enter_context(tc.tile_pool(name="ospool", bufs=3))

            for b in range(NCH):
                qs = slice(QB * b, QB * (b + 1))
                nkt = 4 * b + 4
                for m in range(2):
                    po = [popool.tile([128, QB], F32, name=f"po{hh}_{m}_{b}",
                                      tag=f"po{hh}") for hh in range(2)]
                    for ki in range(nkt):
                        d = ki - 4 * b
                        qlo = max(0, 128 * d)
                        sc = []
                        for hh in range(2):
                            s_t = scpool.tile([128, QB], F32,
                                              name=f"sc{hh}_{m}_{b}_{ki}",
                                              tag=f"sc{hh}")
                            nc.tensor.matmul(
                                s_t[:, qlo:QB],
                                kp_sb[2 * m + hh][:, 128 * ki:128 * (ki + 1)],
                                qT_sb[m][:, QB * b + qlo:QB * (b + 1)],
                                start=True, stop=True)
                            sc.append(s_t)
                        pt = []
                        for hh in range(2):
                            if d >= 0:
                                nc.vector.tensor_tensor(
                                    sc[hh][:, qlo:qlo + 128],
                                    sc[hh][:, qlo:qlo + 128], tri[:], AOP.add)
                            p_t = ptpool.tile([128, QB], CDT,
                                              name=f"pt{hh}_{m}_{b}_{ki}",
                                              tag=f"pt{hh}")
                            nc.scalar.activation(p_t[:, qlo:QB], sc[hh][:, qlo:QB],
                                                 AF.Exp, scale=0.125)
                            pt.append(p_t)
                        for hh in range(2):
                            h = 2 * m + hh
                            nc.tensor.matmul(
                                po[hh][:, qlo:QB],
                                v_sb[ki][:, 128 * h:128 * (h + 1)],
                                pt[hh][:, qlo:QB],
                                start=(ki == 0), stop=(ki == nkt - 1))
                    # normalize by the accumulated denominator row
                    for hh in range(2):
                        hrow = slice(64 * hh, 64 * hh + 64)
                        lsum = nrmpool.tile([1, QB], F32, tag=f"lsum{hh}")
                        rsum = nrmpool.tile([1, QB], F32, tag=f"rsum{hh}")
                        rbc = nrmpool.tile([DH, QB], F32, tag=f"rbc{hh}")
                        nc.scalar.activation(lsum[:], po[hh][DH:DH + 1, :],
                                             AF.Ln)
                        nc.scalar.activation(rsum[:], lsum[:], AF.Exp,
                                             scale=-1.0)
                        nc.gpsimd.partition_broadcast(rbc[:], rsum[:])
                        nc.vector.tensor_tensor(
                            attn_sb[m][hrow, qs], po[hh][0:DH, :], rbc[:],
                            AOP.mult)

            # out-projection, decoupled from the attention loop
            for sq in range(NKT):
                ssl = slice(128 * sq, 128 * (sq + 1))
                ot = ospool.tile([128, DM], F32, tag="ot")
                for on in range(2):
                    osl = slice(512 * on, 512 * (on + 1))
                    ops = opool.tile([128, 512], F32, tag="ops")
                    for ct in range(2):
                        nc.tensor.matmul(ops[:], attn_sb[ct][:, ssl],
                                         wo_t[ct][:, osl],
                                         start=(ct == 0), stop=(ct == 1))
                    nc.scalar.copy(ot[:, osl], ops[:])
                nc.gpsimd.dma_start(out_ext[ssl, :], ot[:])

    nc.compile()
    return nc


_NC_CACHE = []


def kernel(x, Wq, Wk, Wv, Wo):
    x = np.asarray(x, dtype=np.float32)
    Wq = np.asarray(Wq, dtype=np.float32)
    Wk = np.asarray(Wk, dtype=np.float32)
    Wv = np.asarray(Wv, dtype=np.float32)
    Wo = np.asarray(Wo, dtype=np.float32)

    np_cdt = ml_dtypes.bfloat16 if CDT == BF16 else np.float32
    in_maps = []
    for c in range(8):
        b, hg = divmod(c, 4)
        rows = slice(CH * hg, CH * (hg + 1))
        in_maps.append({
            "xT": np.ascontiguousarray(x[b].T).astype(np_cdt),
            "wqT": np.ascontiguousarray(Wq[rows, :].T).astype(np_cdt),
            "wkT": np.ascontiguousarray(Wk[rows, :].T).astype(np_cdt),
            "wvT": np.ascontiguousarray(Wv[rows, :].T).astype(np_cdt),
            "woT": np.ascontiguousarray(Wo[:, rows].T).astype(np_cdt),
        })

    if not _NC_CACHE:
        _NC_CACHE.append(_build())
    nc = _NC_CACHE[0]

    res = run_bass_kernel_spmd(nc, in_maps, list(range(8)))
    out = np.zeros((2, S, DM), dtype=np.float32)
    for c in range(8):
        out[c // 4] += res.results[c]["out"]
    return out
